# revision 5
# baseline (speedup 1.0000x reference)
"""DAEGC-style GNN (AE + 4 GAT layers + GNN + self-similarity diffusion +
student-t assignment) on 8 Trainium2 NeuronCores.

Sharding: nodes (N=4096) split into 8 row-blocks of 512. Each core holds its
rows of x/adj/M (transposed on host) and computes its attention rows; the
per-layer h is all-gathered row-major with the f_neigh column piggybacked on
the gather buffer. Weights replicated.

Device layout: activations feature-major ([feat, local-node], feat on
partitions); attention matrices transposed per core ([j, i-local], j on
partitions in 32 chunks). Softmax-over-j denominators via a ones-column
matmul on the PE; attention needs no PE transposes and its output lands
feature-major, closing the loop for the next projection.

Numerics: fp32r (TF32-like full-speed PE mode) everywhere except the chain
feeding `predict` logits (e_s = h1@h1.T, the g4 layer, g5/adjm/z3 matmuls),
which runs in plain fp32. A 13-bit-mantissa emulation of this split keeps
all outputs within ~1e-4 of fp32 except predict ~6e-4 (scale-relative).
"""

import contextlib

import numpy as np

import concourse.bass_isa as bass_isa
import concourse.mybir as mybir
import concourse.tile as tile
from concourse import bacc
from concourse.bass_utils import run_bass_kernel_spmd

P = 128
NCORES = 8
N = 4096
R = N // NCORES          # 512 local rows
RC = R // P              # 4
JC = N // P              # 32
NIN = 1024
E1, E2, E3, NZ, NCL = 512, 512, 2048, 64, 16
SIGMA = 0.1
ALPHA = 0.2
NEGB = -50.0

F32 = mybir.dt.float32
F32R = mybir.dt.float32r
BF16 = mybir.dt.bfloat16
AF = mybir.ActivationFunctionType
OP = mybir.AluOpType

_CACHED = {}


def _cdiv(a, b):
    return (a + b - 1) // b


def build_nc():
    nc = bacc.Bacc("TRN2", target_bir_lowering=False, debug=False,
                   enable_asserts=True, num_devices=NCORES)
    dt_in = {}

    def din(name, shape, dtype):
        t = nc.dram_tensor(name, list(shape), dtype, kind="ExternalInput")
        dt_in[name] = t
        return t

    xT = din("xT", [NIN, R], F32R)
    adjT = din("adjT", [N, R], F32)
    MT = din("MT", [N, R], F32)

    for nm, di, do in [("enc1", NIN, E1), ("enc2", E1, E2), ("enc3", E2, E3),
                       ("zl", E3, NZ), ("dec1", P, 2048), ("dec2", 2048, E2),
                       ("dec3", E2, 512), ("xbar", 512, NIN)]:
        din(nm + "_w", [di, do], F32R)
        din(nm + "_b", [P, _cdiv(do, P)], F32)
    din("g1_w", [NIN, E1], F32R)
    din("g2_w", [E1, E2], F32R)
    din("g3_w", [E2, E3], F32R)
    din("g4_w", [E3, NZ], F32)
    din("as1", [E1, 1], F32R); din("an1", [E1, 1], F32R)
    din("as2", [E2, 1], F32R); din("an2", [E2, 1], F32R)
    din("as3", [E3, 1], F32R); din("an3", [E3, 1], F32R)
    din("as4", [P, 1], F32); din("an4", [P, 1], F32)
    din("g5w", [P, NCL], F32)
    din("cluT", [P, NCL], F32R)
    din("clun2", [NCL, 1], F32)
    din("identf", [P, P], F32)
    din("identr", [P, P], F32R)
    din("ones_r", [P, 1], F32R)
    din("ones_f", [P, 1], F32)
    din("zer64r", [NZ, R], F32R)
    din("zer64f", [NZ, R], F32)

    xbarT_o = nc.dram_tensor("xbarT", [NIN, R], F32, kind="ExternalOutput")
    zT_o = nc.dram_tensor("zT", [NZ, R], F32, kind="ExternalOutput")
    h4T_o = nc.dram_tensor("h4T", [NZ, R], F32, kind="ExternalOutput")
    qT_o = nc.dram_tensor("qT", [NCL, R], F32, kind="ExternalOutput")
    predT_o = nc.dram_tensor("predT", [NCL, R], F32, kind="ExternalOutput")

    rg = [list(range(NCORES))]

    with tile.TileContext(nc) as tc, contextlib.ExitStack() as ctx:
        cst = ctx.enter_context(tc.tile_pool(name="cst", bufs=1))
        glob = ctx.enter_context(tc.tile_pool(name="glob", bufs=1))
        wp = ctx.enter_context(tc.tile_pool(name="wp", bufs=1))
        tp = ctx.enter_context(tc.tile_pool(name="tp", bufs=3))
        ps = ctx.enter_context(tc.tile_pool(name="ps", bufs=8, space="PSUM"))
        dr = ctx.enter_context(tc.tile_pool(name="dr", bufs=1, space="DRAM"))

        psn = [0]

        def psum(nm="p", dtype=F32):
            psn[0] += 1
            return ps.tile([P, R], dtype, tag="ps", bufs=8,
                           name=f"ps_{nm}{psn[0]}")

        def ttile(nm, tag="t", bufs=8, dtype=F32, shape=None):
            return tp.tile(shape or [P, R], dtype, tag=tag, bufs=bufs, name=nm)

        # ---------- constants ----------
        identf = cst.tile([P, P], F32, tag="identf")
        nc.sync.dma_start(identf[:], dt_in["identf"].ap())
        identr = cst.tile([P, P], F32R, tag="identr")
        nc.sync.dma_start(identr[:], dt_in["identr"].ap())
        ones_r = cst.tile([P, 1], F32R, tag="ones_r")
        nc.sync.dma_start(ones_r[:], dt_in["ones_r"].ap())
        ones_f = cst.tile([P, 1], F32, tag="ones_f")
        nc.sync.dma_start(ones_f[:], dt_in["ones_f"].ap())
        clun2 = cst.tile([NCL, 1], F32, tag="clun2")
        nc.sync.dma_start(clun2[:], dt_in["clun2"].ap())

        def load_vec(name, d, dtype):
            kcn = _cdiv(d, P)
            v = cst.tile([P, kcn, 1], dtype, tag=f"v_{name}")
            nc.sync.dma_start(
                v[:], dt_in[name].ap().rearrange("(o p) one -> p o one", p=P))
            return v

        def load_bias(name, do):
            b = cst.tile([P, _cdiv(do, P)], F32, tag=f"b_{name}")
            nc.sync.dma_start(b[:], dt_in[name].ap())
            return b

        def load_w(name, K, D, dtype=F32R):
            w = wp.tile([P, _cdiv(K, P), D], dtype, tag="w", name=f"w_{name}")
            nc.sync.dma_start(
                w[:], dt_in[name].ap().rearrange("(o p) f -> p o f", p=P))
            return w

        # ---------- prologue ----------
        nb = glob.tile([P, JC, R], BF16, tag="nb")
        for jc in range(JC):
            atj = ttile(f"adjc{jc}")
            nc.sync.dma_start(atj[:], adjT.ap()[jc * P:(jc + 1) * P, :])
            nc.vector.tensor_scalar(nb[:, jc], atj[:], 0.5, NEGB,
                                    OP.is_le, OP.mult)
        xt = glob.tile([P, NIN // P, R], F32R, tag="xt")
        nc.sync.dma_start(xt[:], xT.ap().rearrange("(o p) f -> p o f", p=P))

        # resident activations
        t1t = glob.tile([P, E1 // P, R], F32R, tag="t1t")
        t2t = glob.tile([P, E2 // P, R], F32R, tag="t2t")
        zt = glob.tile([P, R], F32R, tag="zt")
        h1ot = glob.tile([P, E1 // P, R], F32, tag="h1ot")
        bl5 = glob.tile([P, R], F32, tag="bl5")
        nrm2 = glob.tile([1, R], F32, tag="nrm2")

        t3_dram = dr.tile([E3, R], F32R, name="t3_dram")
        d1_dram = dr.tile([2048, R], F32R, name="d1_dram")
        bl4_dram = dr.tile([E3, R], F32, name="bl4_dram")
        pt_dram = dr.tile([N, R], F32R, name="pt_dram")

        def proj_res(rhs, kcn, w, D, out_cb):
            """mc-outer projection with SBUF-resident rhs [P, kcn, R]."""
            for mc in range(_cdiv(D, P)):
                m = min(P, D - mc * P)
                pp = psum("pr")
                for kc in range(kcn):
                    nc.tensor.matmul(pp[:m], w[:, kc, mc * P:mc * P + m],
                                     rhs[:, kc], start=(kc == 0),
                                     stop=(kc == kcn - 1),
                                     skip_group_check=True)
                out_cb(mc, m, pp)

        def proj_stream(src_dram, kcn, w, D, out_cb, src_dt=F32R, nm="pk"):
            """kc-outer projection streaming rhs chunks from DRAM."""
            mcn = _cdiv(D, P)
            pps = [psum(f"{nm}{mc}") for mc in range(mcn)]
            for kc in range(kcn):
                ch = ttile(f"{nm}ch{kc}", dtype=src_dt)
                nc.sync.dma_start(ch[:], src_dram[kc * P:(kc + 1) * P, :])
                for mc in range(mcn):
                    m = min(P, D - mc * P)
                    nc.tensor.matmul(pps[mc][:m], w[:, kc, mc * P:mc * P + m],
                                     ch[:], start=(kc == 0),
                                     stop=(kc == kcn - 1),
                                     skip_group_check=True)
            for mc in range(mcn):
                out_cb(mc, min(P, D - mc * P), pps[mc])

        # ---------- AE ----------
        with nc.named_scope("ae"):
            w = load_w("enc1_w", NIN, E1); b = load_bias("enc1_b", E1)
            proj_res(xt, NIN // P, w, E1, lambda mc, m, pp: nc.scalar.activation(
                t1t[:m, mc], pp[:m], AF.Relu, bias=b[:m, mc:mc + 1]))
            w = load_w("enc2_w", E1, E2); b = load_bias("enc2_b", E2)
            proj_res(t1t, E1 // P, w, E2, lambda mc, m, pp: nc.scalar.activation(
                t2t[:m, mc], pp[:m], AF.Relu, bias=b[:m, mc:mc + 1]))
            w = load_w("enc3_w", E2, E3); b = load_bias("enc3_b", E3)

            def t3_cb(mc, m, pp, b=b):
                t3c = ttile(f"t3c{mc}", dtype=F32R)
                nc.scalar.activation(t3c[:m], pp[:m], AF.Relu,
                                     bias=b[:m, mc:mc + 1])
                nc.sync.dma_start(t3_dram[mc * P:mc * P + m, :], t3c[:m])
            proj_res(t2t, E2 // P, w, E3, t3_cb)

            w = load_w("zl_w", E3, NZ); b = load_bias("zl_b", NZ)

            def zl_cb(mc, m, pp, b=b):
                nc.scalar.activation(zt[:NZ], pp[:NZ], AF.Identity,
                                     bias=b[:NZ, 0:1])
            proj_stream(t3_dram, E3 // P, w, NZ, zl_cb, nm="zl")
            nc.sync.dma_start(zt[NZ:, :], dt_in["zer64r"].ap())
            nc.sync.dma_start(zT_o.ap(), zt[:NZ].bitcast(F32))

            w = load_w("dec1_w", P, 2048); b = load_bias("dec1_b", 2048)
            zt3 = zt[:, None, :]

            def d1_cb(mc, m, pp, b=b):
                d1c = ttile(f"d1c{mc}", dtype=F32R)
                nc.scalar.activation(d1c[:m], pp[:m], AF.Relu,
                                     bias=b[:m, mc:mc + 1])
                nc.sync.dma_start(d1_dram[mc * P:mc * P + m, :], d1c[:m])
            proj_res(zt3, 1, w, 2048, d1_cb)

            d2t = wp.tile([P, E2 // P, R], F32R, tag="d2t")
            w = load_w("dec2_w", 2048, E2); b = load_bias("dec2_b", E2)
            proj_stream(d1_dram, 2048 // P, w, E2,
                        lambda mc, m, pp, b=b: nc.scalar.activation(
                            d2t[:m, mc], pp[:m], AF.Relu, bias=b[:m, mc:mc + 1]),
                        nm="d2")
            d3t = wp.tile([P, 512 // P, R], F32R, tag="d3t")
            w = load_w("dec3_w", E2, 512); b = load_bias("dec3_b", 512)
            proj_res(d2t, E2 // P, w, 512, lambda mc, m, pp, b=b:
                     nc.scalar.activation(d3t[:m, mc], pp[:m], AF.Relu,
                                          bias=b[:m, mc:mc + 1]))
            w = load_w("xbar_w", 512, NIN); b = load_bias("xbar_b", NIN)

            def xb_cb(mc, m, pp, b=b):
                xbc = ttile(f"xbc{mc}")
                nc.scalar.activation(xbc[:m], pp[:m], AF.Identity,
                                     bias=b[:m, mc:mc + 1])
                nc.sync.dma_start(xbarT_o.ap()[mc * P:mc * P + m, :], xbc[:m])
            proj_res(d3t, 512 // P, w, NIN, xb_cb)

        # ---------- GAT helpers ----------
        def gat_proj_gather(lname, rhs, kcn, wname, d, asn, ann, vdt,
                            h_full=None, wdt=None):
            """Project h (chunkwise), accumulate f_self/f_neigh, transpose each
            chunk into the row-major DRAM bounce, gather. Returns
            (hg, fs_tile, bo)."""
            mcn = _cdiv(d, P)
            w = load_w(wname, kcn * P, d, wdt if wdt is not None else vdt)
            asv = load_vec(asn, d, vdt)
            anv = load_vec(ann, d, vdt)
            fs_ps, fn_ps = psum("fs"), psum("fn")
            bo = dr.tile([R, d + 8], F32R if vdt == F32R else F32,
                         name=f"bo_{lname}")
            idn = identf if vdt == F32 else identr

            for mc in range(mcn):
                m = min(P, d - mc * P)
                pp = psum("hp")
                for kc in range(kcn):
                    nc.tensor.matmul(pp[:m], w[:, kc, mc * P:mc * P + m],
                                     rhs[:, kc], start=(kc == 0),
                                     stop=(kc == kcn - 1),
                                     skip_group_check=True)
                if h_full is not None:
                    hc = h_full[:, mc]
                    nc.vector.tensor_copy(hc, pp[:m])
                else:
                    hc = ttile(f"hc_{lname}{mc}", tag="hc", bufs=2, dtype=vdt)
                    nc.vector.tensor_copy(hc[:m], pp[:m])
                    hc = hc[:m]
                nc.tensor.matmul(fs_ps[:1], asv[:, mc], hc,
                                 start=(mc == 0), stop=(mc == mcn - 1),
                                 skip_group_check=True)
                nc.tensor.matmul(fn_ps[:1], anv[:, mc], hc,
                                 start=(mc == 0), stop=(mc == mcn - 1),
                                 skip_group_check=True)
                for ns in range(RC):
                    tpp = psum("tp", dtype=vdt)
                    nc.tensor.transpose(tpp[:, :P], hc[:, ns * P:(ns + 1) * P],
                                        idn[:])
                    blk = ttile(f"bk_{lname}{mc}_{ns}", tag="blk", bufs=6,
                                dtype=bo.dtype, shape=[P, P])
                    nc.vector.tensor_copy(blk[:].bitcast(F32),
                                          tpp[:, :P].bitcast(F32))
                    nc.sync.dma_start(
                        bo[ns * P:(ns + 1) * P, mc * P:mc * P + m], blk[:, :m])

            fs_t = ttile(f"fs_{lname}", tag="frow", bufs=6, shape=[1, R])
            nc.scalar.copy(fs_t[:], fs_ps[:1])
            fn_t = ttile(f"fn_{lname}", tag="frow", bufs=6, shape=[1, R])
            nc.scalar.copy(fn_t[:], fn_ps[:1])
            bocol = bo[:, d:d + 1]
            if bo.dtype != F32:
                bocol = bocol.bitcast(F32)
            nc.sync.dma_start(bocol.rearrange("f p -> p f"), fn_t[:])
            hg = dr.tile([NCORES, R, d + 8], bo.dtype, name=f"hg_{lname}",
                         addr_space="Shared")
            nc.gpsimd.collective_compute(
                "AllGather", OP.bypass, replica_groups=rg,
                ins=[bo.opt()], outs=[hg.opt()])
            return hg, fs_t

        def gat_attention(lname, hg, d, fs_t, exact, epi, spill):
            """Fused e-chain + att@h. epi(mc, m, y) gets normalized out.T
            chunks. spill=True: 3-pass p spill via pt_dram (for d=2048)."""
            mcn = _cdiv(d, P)
            FS = ttile(f"FS_{lname}", tag="pfull", bufs=3)
            nc.gpsimd.partition_broadcast(FS[:], fs_t[:])
            if spill:
                passes = [list(range(0, 7)), list(range(7, 15)), [15]]
            else:
                passes = [list(range(mcn))]
            rb = None
            for pi, mcs in enumerate(passes):
                pos = {mc: psum(f"po_{lname}{mc}") for mc in mcs}
                rs_ps = psum("rs") if pi == 0 else None
                for jc in range(JC):
                    cc, rr_ = jc // RC, jc % RC
                    if pi == 0:
                        fnc = tp.tile([P, 1], F32, tag="fnc", bufs=4,
                                      name=f"fnc_{lname}{jc}")
                        src = hg[cc, rr_ * P:(rr_ + 1) * P, d:d + 1]
                        if src.dtype != F32:
                            src = src.bitcast(F32)
                        nc.sync.dma_start(fnc[:], src)
                        mtj = ttile(f"mt_{lname}{jc}")
                        nc.sync.dma_start(mtj[:],
                                          MT.ap()[jc * P:(jc + 1) * P, :])
                        u = ttile(f"u_{lname}{jc}")
                        nc.vector.scalar_tensor_tensor(
                            u[:], FS[:], fnc[:], mtj[:], OP.add, OP.mult)
                        el = ttile(f"el_{lname}{jc}")
                        nc.scalar.activation(el[:], u[:], AF.Prelu, alpha=ALPHA)
                        em = ttile(f"em_{lname}{jc}")
                        nc.vector.tensor_add(em[:], el[:], nb[:, jc])
                        pch = ttile(f"pch_{lname}{jc}", tag="pch", bufs=4,
                                    dtype=F32R)
                        nc.scalar.activation(pch[:], em[:], AF.Exp)
                        if spill:
                            nc.sync.dma_start(
                                pt_dram[jc * P:(jc + 1) * P, :], pch[:])
                        nc.tensor.matmul(rs_ps[:1], ones_r[:], pch[:],
                                         start=(jc == 0), stop=(jc == JC - 1),
                                         skip_group_check=True)
                    else:
                        pch = ttile(f"pch_{lname}{pi}_{jc}", tag="pch", bufs=4,
                                    dtype=F32R)
                        nc.sync.dma_start(pch[:],
                                          pt_dram[jc * P:(jc + 1) * P, :])
                    rhs = pch[:].bitcast(F32) if exact else pch[:]
                    for mc in mcs:
                        m = min(P, d - mc * P)
                        hb = tp.tile([P, P], F32 if exact else F32R, tag="blk",
                                     bufs=6, name=f"hb_{lname}{pi}_{mc}_{jc}")
                        hsrc = hg[cc, rr_ * P:(rr_ + 1) * P,
                                  mc * P:mc * P + m]
                        if hsrc.dtype != hb.dtype:
                            hsrc = hsrc.bitcast(hb.dtype)
                        nc.sync.dma_start(hb[:, :m], hsrc)
                        nc.tensor.matmul(pos[mc][:m], hb[:, :m], rhs,
                                         start=(jc == 0), stop=(jc == JC - 1),
                                         skip_group_check=True)
                if pi == 0:
                    rs = ttile(f"rs_{lname}", tag="frow", bufs=6, shape=[1, R])
                    nc.scalar.copy(rs[:], rs_ps[:1])
                    rr2 = ttile(f"rr_{lname}", tag="frow", bufs=6,
                                shape=[1, R])
                    nc.vector.reciprocal(rr2[:], rs[:])
                    rb = ttile(f"rb_{lname}", tag="pfull", bufs=3)
                    nc.gpsimd.partition_broadcast(rb[:], rr2[:])
                for mc in mcs:
                    m = min(P, d - mc * P)
                    y = ttile(f"y_{lname}{mc}")
                    nc.vector.tensor_mul(y[:m], pos[mc][:m], rb[:m])
                    epi(mc, m, y)

        def elu_c1(lname, mc, m, y):
            mn = ttile(f"mn_{lname}{mc}")
            nc.vector.tensor_scalar_min(mn[:m], y[:m], 0.0)
            ex = ttile(f"ex_{lname}{mc}")
            nc.scalar.activation(ex[:m], mn[:m], AF.Exp)
            c1 = ttile(f"c1_{lname}{mc}")
            nc.vector.scalar_tensor_tensor(
                c1[:m], y[:m], 0.0, ex[:m], OP.max, OP.add)
            return c1  # relu(y) + exp(min(y,0)) ; elu = c1 - 1

        def blend_ip(lname, dst, tsrc, c1, mc, m):
            # dst = 0.9*(c1-1) + 0.1*tsrc
            c2 = ttile(f"c2_{lname}{mc}")
            nc.vector.tensor_scalar(c2[:m], c1[:m], -1.0, 0.9, OP.add, OP.mult)
            nc.vector.scalar_tensor_tensor(dst, tsrc, 0.1, c2[:m],
                                           OP.mult, OP.add)

        # ---------- g1 ----------
        with nc.named_scope("g1"):
            hg1, fs1 = gat_proj_gather("g1", xt, NIN // P, "g1_w", E1,
                                       "as1", "an1", F32R)

            def epi1(mc, m, y):
                c1 = elu_c1("g1", mc, m, y)
                # h1 (gat output) kept for the s-diffusion chain
                nc.vector.tensor_scalar_add(h1ot[:m, mc], c1[:m], -1.0)
                c2 = ttile(f"c2g1_{mc}")
                nc.vector.tensor_scalar_mul(c2[:m], h1ot[:m, mc], 0.9)
                nc.vector.scalar_tensor_tensor(
                    t1t[:m, mc], t1t[:m, mc].bitcast(F32), 0.1, c2[:m],
                    OP.mult, OP.add)
            gat_attention("g1", hg1, E1, fs1, False, epi1, False)

            # gather h1 (gat output, feature-major) + per-node sq-norms row
            nr_ps = psum("nr")
            for mc in range(E1 // P):
                sqc = ttile(f"sq{mc}")
                nc.scalar.activation(sqc[:], h1ot[:, mc], AF.Square)
                nc.tensor.matmul(nr_ps[:1], ones_f[:], sqc[:],
                                 start=(mc == 0), stop=(mc == E1 // P - 1),
                                 skip_group_check=True)
            nc.scalar.copy(nrm2[:], nr_ps[:1])
            bo1o = dr.tile([E1 + 8, R], F32, name="bo1o")
            nc.sync.dma_start(bo1o[:E1].rearrange("(o p) f -> p o f", p=P),
                              h1ot[:])
            nc.sync.dma_start(bo1o[E1:E1 + 1, :], nrm2[:])
            hg1o = dr.tile([NCORES, E1 + 8, R], F32, name="hg1o",
                           addr_space="Shared")
            nc.gpsimd.collective_compute(
                "AllGather", OP.bypass, replica_groups=rg,
                ins=[bo1o.opt()], outs=[hg1o.opt()])

        # ---------- g2 ----------
        with nc.named_scope("g2"):
            hg2, fs2 = gat_proj_gather("g2", t1t, E1 // P, "g2_w", E2,
                                       "as2", "an2", F32R)

            def epi2(mc, m, y):
                c1 = elu_c1("g2", mc, m, y)
                blend_ip("g2", t2t[:m, mc], t2t[:m, mc].bitcast(F32), c1, mc, m)
            gat_attention("g2", hg2, E2, fs2, False, epi2, False)

        # ---------- g3 ----------
        with nc.named_scope("g3"):
            hg3, fs3 = gat_proj_gather("g3", t2t, E2 // P, "g3_w", E3,
                                       "as3", "an3", F32R)

            def epi3(mc, m, y):
                c1 = elu_c1("g3", mc, m, y)
                t3e = ttile(f"t3e{mc}", dtype=F32R)
                nc.sync.dma_start(t3e[:m], t3_dram[mc * P:mc * P + m, :])
                c2 = ttile(f"c23_{mc}")
                nc.vector.tensor_scalar(c2[:m], c1[:m], -1.0, 0.9,
                                        OP.add, OP.mult)
                bl = ttile(f"bl3_{mc}")
                nc.vector.scalar_tensor_tensor(
                    bl[:m], t3e[:m].bitcast(F32), 0.1, c2[:m],
                    OP.mult, OP.add)
                nc.sync.dma_start(bl4_dram[mc * P:mc * P + m, :], bl[:m])
            gat_attention("g3", hg3, E3, fs3, False, epi3, True)

        # ---------- g4 (exact fp32) ----------
        with nc.named_scope("g4"):
            w4 = load_w("g4_w", E3, NZ, F32)
            h4t = ttile("h4t", tag="pfull", bufs=3)
            bo4 = dr.tile([R, NZ + 8], F32, name="bo_g4")
            p4t = psum("h4t")
            p4r = [psum(f"h4r{ns}") for ns in range(RC)]
            for kc in range(E3 // P):
                blk4 = ttile(f"bl4k{kc}")
                nc.sync.dma_start(blk4[:], bl4_dram[kc * P:(kc + 1) * P, :])
                nc.tensor.matmul(p4t[:NZ], w4[:, kc, :NZ], blk4[:],
                                 start=(kc == 0), stop=(kc == E3 // P - 1),
                                 skip_group_check=True)
                for ns in range(RC):
                    nc.tensor.matmul(p4r[ns][:, :NZ],
                                     blk4[:, ns * P:(ns + 1) * P],
                                     w4[:, kc, :NZ], start=(kc == 0),
                                     stop=(kc == E3 // P - 1),
                                     skip_group_check=True)
            nc.vector.tensor_copy(h4t[:NZ], p4t[:NZ])
            nc.sync.dma_start(h4t[NZ:], dt_in["zer64f"].ap())
            for ns in range(RC):
                h4rc = tp.tile([P, NZ], F32, tag="blk", bufs=6,
                               name=f"h4rc{ns}")
                nc.vector.tensor_copy(h4rc[:], p4r[ns][:, :NZ])
                nc.sync.dma_start(bo4[ns * P:(ns + 1) * P, :NZ], h4rc[:])
            asv4 = cst.tile([P, 1], F32, tag="as4")
            nc.sync.dma_start(asv4[:], dt_in["as4"].ap())
            anv4 = cst.tile([P, 1], F32, tag="an4")
            nc.sync.dma_start(anv4[:], dt_in["an4"].ap())
            pf4 = psum("fs4")
            nc.tensor.matmul(pf4[:1], asv4[:], h4t[:], start=True, stop=True,
                             skip_group_check=True)
            fs4 = ttile("fs4", tag="frow", bufs=6, shape=[1, R])
            nc.scalar.copy(fs4[:], pf4[:1])
            pn4 = psum("fn4")
            nc.tensor.matmul(pn4[:1], anv4[:], h4t[:], start=True, stop=True,
                             skip_group_check=True)
            fn4 = ttile("fn4", tag="frow", bufs=6, shape=[1, R])
            nc.scalar.copy(fn4[:], pn4[:1])
            nc.sync.dma_start(bo4[:, NZ:NZ + 1].rearrange("f p -> p f"),
                              fn4[:])
            hg4 = dr.tile([NCORES, R, NZ + 8], F32, name="hg_g4",
                          addr_space="Shared")
            nc.gpsimd.collective_compute(
                "AllGather", OP.bypass, replica_groups=rg,
                ins=[bo4.opt()], outs=[hg4.opt()])

            zt01 = ttile("zt01", tag="pfull", bufs=3)
            nc.vector.tensor_scalar_mul(zt01[:NZ], zt[:NZ].bitcast(F32), 0.1)

            def epi4(mc, m, y):
                c1 = elu_c1("g4", mc, m, y)
                h4o = ttile("h4o")
                nc.vector.tensor_scalar_add(h4o[:m], c1[:m], -1.0)
                nc.sync.dma_start(h4T_o.ap(), h4o[:m])
                nc.vector.scalar_tensor_tensor(
                    bl5[:m], h4o[:m], 0.9, zt01[:m], OP.mult, OP.add)
            gat_attention("g4", hg4, NZ, fs4, True, epi4, False)
            nc.sync.dma_start(bl5[NZ:], dt_in["zer64f"].ap())

        # ---------- GNN + s-diffusion ----------
        with nc.named_scope("gnn_s"):
            w5 = cst.tile([P, NCL], F32, tag="g5w")
            nc.sync.dma_start(w5[:], dt_in["g5w"].ap())
            pw = psum("wg")
            nc.tensor.matmul(pw[:NCL], w5[:], bl5[:], start=True, stop=True,
                             skip_group_check=True)
            wgT = ttile("wgT", tag="c16", bufs=6, shape=[NCL, R])
            nc.scalar.copy(wgT[:], pw[:NCL])
            bw = dr.tile([R, NCL], F32, name="bw")
            nc.sync.dma_start(bw[:].rearrange("n c -> c n"), wgT[:])
            wg_row = dr.tile([NCORES, R, NCL], F32, name="wg_row",
                             addr_space="Shared")
            nc.gpsimd.collective_compute(
                "AllGather", OP.bypass, replica_groups=rg,
                ins=[bw.opt()], outs=[wg_row.opt()])

            ph = psum("hgnn")
            for jc in range(JC):
                cc, rr_ = jc // RC, jc % RC
                wrb = tp.tile([P, NCL], F32, tag="blk", bufs=6,
                              name=f"wrb{jc}")
                nc.sync.dma_start(wrb[:], wg_row[cc, rr_ * P:(rr_ + 1) * P, :])
                atj = ttile(f"adj2_{jc}")
                nc.sync.dma_start(atj[:], adjT.ap()[jc * P:(jc + 1) * P, :])
                adm = ttile(f"adm{jc}")
                nc.vector.scalar_tensor_tensor(
                    adm[:], atj[:], 0.5, atj[:], OP.is_gt, OP.mult)
                nc.tensor.matmul(ph[:NCL], wrb[:], adm[:], start=(jc == 0),
                                 stop=(jc == JC - 1), skip_group_check=True)
            hgt = ttile("hgt", tag="c16", bufs=6, shape=[NCL, R])
            nc.scalar.copy(hgt[:], ph[:NCL])
            bh = dr.tile([R, NCL], F32, name="bh")
            nc.sync.dma_start(bh[:].rearrange("n c -> c n"), hgt[:])
            hg_gnn = dr.tile([NCORES, R, NCL], F32, name="hg_gnn",
                             addr_space="Shared")
            nc.gpsimd.collective_compute(
                "AllGather", OP.bypass, replica_groups=rg,
                ins=[bh.opt()], outs=[hg_gnn.opt()])

            # K_i = sqrt(nrm2_i) * maxnorm for a safe exp
            mx = tp.tile([1, 1], F32, tag="mx", bufs=1, name="mx")
            mxc = tp.tile([1, 1], F32, tag="mxc", bufs=8, name="mxc")
            for c in range(NCORES):
                nr_c = ttile(f"nr{c}", tag="frow", bufs=6, shape=[1, R])
                nc.sync.dma_start(nr_c[:], hg1o[c, E1:E1 + 1, :])
                red = tp.tile([1, 1], F32, tag="mxc", bufs=8, name=f"red{c}")
                nc.vector.tensor_reduce(red[:], nr_c[:], mybir.AxisListType.X,
                                        OP.max)
                if c == 0:
                    nc.vector.tensor_copy(mx[:], red[:])
                else:
                    nc.vector.tensor_tensor(mx[:], mx[:], red[:], OP.max)
            mxs = tp.tile([1, 1], F32, tag="mxs", bufs=1, name="mxs")
            nc.scalar.activation(mxs[:], mx[:], AF.Sqrt)
            kr = ttile("kr", tag="frow", bufs=6, shape=[1, R])
            nc.scalar.activation(kr[:], nrm2[:], AF.Sqrt)
            nc.vector.tensor_scalar(kr[:], kr[:], mxs[:], None, OP.mult)
            kb = ttile("kb", tag="pfull", bufs=3)
            nc.gpsimd.partition_broadcast(kb[:], kr[:])

            pdn = psum("dn")
            pz3 = psum("z3")
            for jc in range(JC):
                cc, rr_ = jc // RC, jc % RC
                pes = psum(f"es{jc}")
                for kc in range(E1 // P):
                    h1b = tp.tile([P, P], F32, tag="blk", bufs=6,
                                  name=f"h1b_{jc}_{kc}")
                    nc.sync.dma_start(
                        h1b[:], hg1o[cc, kc * P:(kc + 1) * P,
                                     rr_ * P:(rr_ + 1) * P])
                    nc.tensor.matmul(pes[:], h1b[:], h1ot[:, kc],
                                     start=(kc == 0), stop=(kc == E1 // P - 1),
                                     skip_group_check=True)
                esm = ttile(f"esm{jc}")
                nc.vector.tensor_tensor(esm[:], pes[:], kb[:], OP.subtract)
                psc = ttile(f"psc{jc}")
                nc.scalar.activation(psc[:], esm[:], AF.Exp)
                nc.tensor.matmul(pdn[:1], ones_f[:], psc[:], start=(jc == 0),
                                 stop=(jc == JC - 1), skip_group_check=True)
                hgb = tp.tile([P, NCL], F32, tag="blk", bufs=6,
                              name=f"hgb{jc}")
                nc.sync.dma_start(hgb[:], hg_gnn[cc, rr_ * P:(rr_ + 1) * P, :])
                nc.tensor.matmul(pz3[:NCL], hgb[:], psc[:], start=(jc == 0),
                                 stop=(jc == JC - 1), skip_group_check=True)
            dnb = ttile("dnb", tag="frow", bufs=6, shape=[1, R])
            nc.scalar.copy(dnb[:], pdn[:1])
            dnr = ttile("dnr", tag="frow", bufs=6, shape=[1, R])
            nc.vector.reciprocal(dnr[:], dnb[:])
            drb = ttile("drb", tag="c16", bufs=6, shape=[NCL, R])
            nc.gpsimd.partition_broadcast(drb[:], dnr[:], channels=NCL)
            z3n = ttile("z3n", tag="c16", bufs=6, shape=[NCL, R])
            nc.vector.tensor_mul(z3n[:], pz3[:NCL], drb[:])

            mxp = ttile("mxp", tag="c16", bufs=6, shape=[NCL, R])
            nc.gpsimd.partition_all_reduce(mxp[:], z3n[:], NCL,
                                           bass_isa.ReduceOp.max)
            zc = ttile("zc", tag="c16", bufs=6, shape=[NCL, R])
            nc.vector.tensor_tensor(zc[:], z3n[:], mxp[:], OP.subtract)
            pe_ = ttile("pe", tag="c16", bufs=6, shape=[NCL, R])
            nc.scalar.activation(pe_[:], zc[:], AF.Exp)
            se_ = ttile("se", tag="c16", bufs=6, shape=[NCL, R])
            nc.gpsimd.partition_all_reduce(se_[:], pe_[:], NCL,
                                           bass_isa.ReduceOp.add)
            sre = ttile("sre", tag="c16", bufs=6, shape=[NCL, R])
            nc.vector.reciprocal(sre[:], se_[:])
            prd = ttile("prd", tag="c16", bufs=6, shape=[NCL, R])
            nc.vector.tensor_mul(prd[:], pe_[:], sre[:])
            nc.sync.dma_start(predT_o.ap(), prd[:])

        # ---------- student-t q ----------
        with nc.named_scope("studentt"):
            clu = cst.tile([P, NCL], F32R, tag="cluT")
            nc.sync.dma_start(clu[:], dt_in["cluT"].ap())
            pcr = psum("cross")
            nc.tensor.matmul(pcr[:NCL], clu[:], zt[:], start=True, stop=True,
                             skip_group_check=True)
            zsq = ttile("zsq")
            nc.scalar.activation(zsq[:], zt[:].bitcast(F32), AF.Square)
            pzn = psum("zn")
            nc.tensor.matmul(pzn[:1], ones_f[:], zsq[:], start=True, stop=True,
                             skip_group_check=True)
            znl = ttile("znl", tag="frow", bufs=6, shape=[1, R])
            nc.scalar.copy(znl[:], pzn[:1])
            znb = ttile("znb", tag="c16", bufs=6, shape=[NCL, R])
            nc.gpsimd.partition_broadcast(znb[:], znl[:], channels=NCL)
            d2 = ttile("d2", tag="c16", bufs=6, shape=[NCL, R])
            nc.vector.scalar_tensor_tensor(
                d2[:], pcr[:NCL], -2.0, znb[:], OP.mult, OP.add)
            nc.vector.tensor_scalar(d2[:], d2[:], clun2[:], 1.0,
                                    OP.add, OP.add)
            qn = ttile("qn", tag="c16", bufs=6, shape=[NCL, R])
            nc.vector.reciprocal(qn[:], d2[:])
            qs = ttile("qs", tag="c16", bufs=6, shape=[NCL, R])
            nc.gpsimd.partition_all_reduce(qs[:], qn[:], NCL,
                                           bass_isa.ReduceOp.add)
            qsr = ttile("qsr", tag="c16", bufs=6, shape=[NCL, R])
            nc.vector.reciprocal(qsr[:], qs[:])
            qf = ttile("qf", tag="c16", bufs=6, shape=[NCL, R])
            nc.vector.tensor_mul(qf[:], qn[:], qsr[:])
            nc.sync.dma_start(qT_o.ap(), qf[:])

    nc.compile()
    return nc


def _prep_maps(x, adj, M, params):
    p = {k: np.ascontiguousarray(np.asarray(v, dtype=np.float32))
         for k, v in params.items()}
    x = np.asarray(x, dtype=np.float32)
    adj = np.asarray(adj, dtype=np.float32)
    M = np.asarray(M, dtype=np.float32)

    shared = {}
    for nm in ["enc1", "enc2", "enc3", "zl", "dec2", "dec3", "xbar"]:
        shared[nm + "_w"] = p[nm + "_w"]
    shared["dec1_w"] = np.pad(p["dec1_w"], ((0, P - NZ), (0, 0)))
    for nm, do in [("enc1", E1), ("enc2", E2), ("enc3", E3), ("zl", NZ),
                   ("dec1", 2048), ("dec2", E2), ("dec3", 512), ("xbar", NIN)]:
        b = p[nm + "_b"]
        if do >= P:
            shared[nm + "_b"] = np.ascontiguousarray(b.reshape(-1, P).T)
        else:
            bb = np.zeros((P, 1), np.float32)
            bb[:do, 0] = b
            shared[nm + "_b"] = bb
    for i in (1, 2, 3, 4):
        shared[f"g{i}_w"] = p[f"g{i}_w"]
        a_s, a_n = p[f"g{i}_as"], p[f"g{i}_an"]
        if a_s.shape[0] < P:
            a_s = np.pad(a_s, ((0, P - a_s.shape[0]), (0, 0)))
            a_n = np.pad(a_n, ((0, P - a_n.shape[0]), (0, 0)))
        shared[f"as{i}"] = a_s
        shared[f"an{i}"] = a_n
    shared["g5w"] = np.pad(p["g5_w"], ((0, P - NZ), (0, 0)))
    shared["cluT"] = np.pad(np.ascontiguousarray(p["cluster"].T),
                            ((0, P - NZ), (0, 0)))
    shared["clun2"] = np.ascontiguousarray(
        (p["cluster"] ** 2).sum(axis=1, keepdims=True))
    shared["identf"] = np.eye(P, dtype=np.float32)
    shared["identr"] = np.eye(P, dtype=np.float32)
    shared["ones_r"] = np.ones((P, 1), np.float32)
    shared["ones_f"] = np.ones((P, 1), np.float32)
    shared["zer64r"] = np.zeros((NZ, R), np.float32)
    shared["zer64f"] = np.zeros((NZ, R), np.float32)

    in_maps = []
    for c in range(NCORES):
        r0, r1 = c * R, (c + 1) * R
        m = dict(shared)
        m["xT"] = np.ascontiguousarray(x[r0:r1].T)
        m["adjT"] = np.ascontiguousarray(adj[r0:r1].T)
        m["MT"] = np.ascontiguousarray(M[r0:r1].T)
        in_maps.append(m)
    return in_maps


def kernel(x, adj, M, params):
    if "nc" not in _CACHED:
        _CACHED["nc"] = build_nc()
    nc = _CACHED["nc"]
    in_maps = _prep_maps(x, adj, M, params)
    res = run_bass_kernel_spmd(nc, in_maps, core_ids=list(range(NCORES)))
    xb = np.concatenate([r["xbarT"].T for r in res.results], axis=0)
    q = np.concatenate([r["qT"].T for r in res.results], axis=0)
    pred = np.concatenate([r["predT"].T for r in res.results], axis=0)
    z = np.concatenate([r["zT"].T for r in res.results], axis=0)
    h4 = np.concatenate([r["h4T"].T for r in res.results], axis=0)
    return (xb, q, pred, z, h4)


# revision 8
# speedup vs baseline: 1.0323x; 1.0323x over previous
"""DAEGC-style GNN (AE + 4 GAT layers + GNN + self-similarity diffusion +
student-t assignment) on 8 Trainium2 NeuronCores.

Sharding: nodes (N=4096) split into 8 row-blocks of 512. Each core holds its
rows of x/adj/M (transposed on host) and computes its attention rows; the
per-layer h is all-gathered row-major with the f_neigh column piggybacked on
the gather buffer. Weights replicated.

Device layout: activations feature-major ([feat, local-node], feat on
partitions); attention matrices transposed per core ([j, i-local], j on
partitions in 32 chunks). Softmax-over-j denominators via a ones-column
matmul on the PE; attention needs no PE transposes and its output lands
feature-major, closing the loop for the next projection.

Numerics: fp32r (TF32-like full-speed PE mode) everywhere except the chain
feeding `predict` logits (e_s = h1@h1.T, the g4 layer, g5/adjm/z3 matmuls),
which runs in plain fp32. A 13-bit-mantissa emulation of this split keeps
all outputs within ~1e-4 of fp32 except predict ~6e-4 (scale-relative).
"""

import contextlib

import numpy as np

import concourse.bass_isa as bass_isa
import concourse.mybir as mybir
import concourse.tile as tile
from concourse import bacc
from concourse.bass_utils import run_bass_kernel_spmd

P = 128
NCORES = 8
N = 4096
R = N // NCORES          # 512 local rows
RC = R // P              # 4
JC = N // P              # 32
NIN = 1024
E1, E2, E3, NZ, NCL = 512, 512, 2048, 64, 16
SIGMA = 0.1
ALPHA = 0.2
NEGB = -50.0

F32 = mybir.dt.float32
F32R = mybir.dt.float32r
BF16 = mybir.dt.bfloat16
AF = mybir.ActivationFunctionType
OP = mybir.AluOpType

_CACHED = {}


def _cdiv(a, b):
    return (a + b - 1) // b


def build_nc():
    nc = bacc.Bacc("TRN2", target_bir_lowering=False, debug=False,
                   enable_asserts=True, num_devices=NCORES)
    dt_in = {}

    def din(name, shape, dtype):
        t = nc.dram_tensor(name, list(shape), dtype, kind="ExternalInput")
        dt_in[name] = t
        return t

    xT = din("xT", [NIN, R], F32R)
    adjT = din("adjT", [N, R], F32)
    MT = din("MT", [N, R], F32)

    for nm, di, do in [("enc1", NIN, E1), ("enc2", E1, E2), ("enc3", E2, E3),
                       ("zl", E3, NZ), ("dec1", P, 2048), ("dec2", 2048, E2),
                       ("dec3", E2, 512), ("xbar", 512, NIN)]:
        din(nm + "_w", [di, do], F32R)
        din(nm + "_b", [P, _cdiv(do, P)], F32)
    din("g1_w", [NIN, E1], F32R)
    din("g2_w", [E1, E2], F32R)
    din("g3_w", [E2, E3], F32R)
    din("g4_w", [E3, NZ], F32)
    din("as1", [E1, 1], F32R); din("an1", [E1, 1], F32R)
    din("as2", [E2, 1], F32R); din("an2", [E2, 1], F32R)
    din("as3", [E3, 1], F32R); din("an3", [E3, 1], F32R)
    din("as4", [P, 1], F32); din("an4", [P, 1], F32)
    din("g5w", [P, NCL], F32)
    din("cluT", [P, NCL], F32R)
    din("clun2", [NCL, 1], F32)
    din("identf", [P, P], F32)
    din("identr", [P, P], F32R)
    din("ones_r", [P, 1], F32R)
    din("ones_f", [P, 1], F32)
    din("zer64r", [NZ, R], F32R)
    din("zer64f", [NZ, R], F32)

    xbarT_o = nc.dram_tensor("xbarT", [NIN, R], F32, kind="ExternalOutput")
    zT_o = nc.dram_tensor("zT", [NZ, R], F32, kind="ExternalOutput")
    h4T_o = nc.dram_tensor("h4T", [NZ, R], F32, kind="ExternalOutput")
    qT_o = nc.dram_tensor("qT", [NCL, R], F32, kind="ExternalOutput")
    predT_o = nc.dram_tensor("predT", [NCL, R], F32, kind="ExternalOutput")

    rg = [list(range(NCORES))]

    with tile.TileContext(nc) as tc, contextlib.ExitStack() as ctx:
        cst = ctx.enter_context(tc.tile_pool(name="cst", bufs=1))
        glob = ctx.enter_context(tc.tile_pool(name="glob", bufs=1))
        wp = ctx.enter_context(tc.tile_pool(name="wp", bufs=1))
        tp = ctx.enter_context(tc.tile_pool(name="tp", bufs=3))
        ps = ctx.enter_context(tc.tile_pool(name="ps", bufs=8, space="PSUM"))
        dr = ctx.enter_context(tc.tile_pool(name="dr", bufs=1, space="DRAM"))

        psn = [0]

        def psum(nm="p", dtype=F32):
            psn[0] += 1
            return ps.tile([P, R], dtype, tag="ps", bufs=8,
                           name=f"ps_{nm}{psn[0]}")

        def ttile(nm, tag="t", bufs=5, dtype=F32, shape=None):
            return tp.tile(shape or [P, R], dtype, tag=tag, bufs=bufs, name=nm)

        # ---------- constants ----------
        identf = cst.tile([P, P], F32, tag="identf")
        nc.sync.dma_start(identf[:], dt_in["identf"].ap())
        identr = cst.tile([P, P], F32R, tag="identr")
        nc.sync.dma_start(identr[:], dt_in["identr"].ap())
        ones_r = cst.tile([P, 1], F32R, tag="ones_r")
        nc.sync.dma_start(ones_r[:], dt_in["ones_r"].ap())
        ones_f = cst.tile([P, 1], F32, tag="ones_f")
        nc.sync.dma_start(ones_f[:], dt_in["ones_f"].ap())
        clun2 = cst.tile([NCL, 1], F32, tag="clun2")
        nc.sync.dma_start(clun2[:], dt_in["clun2"].ap())

        def load_vec(name, d, dtype):
            kcn = _cdiv(d, P)
            v = cst.tile([P, kcn, 1], dtype, tag=f"v_{name}")
            nc.sync.dma_start(
                v[:], dt_in[name].ap().rearrange("(o p) one -> p o one", p=P))
            return v

        def load_bias(name, do):
            b = cst.tile([P, _cdiv(do, P)], F32, tag=f"b_{name}")
            nc.sync.dma_start(b[:], dt_in[name].ap())
            return b

        def load_w(name, K, D, dtype=F32R):
            w = wp.tile([P, _cdiv(K, P), D], dtype, tag="w", name=f"w_{name}")
            nc.sync.dma_start(
                w[:], dt_in[name].ap().rearrange("(o p) f -> p o f", p=P))
            return w

        # ---------- prologue ----------
        nb = glob.tile([P, JC, R], BF16, tag="nb")
        for j4 in range(JC // 4):
            atj = ttile(f"adjc{j4}", tag="mt4", bufs=2, shape=[P, 4, R])
            nc.sync.dma_start(
                atj[:], adjT.ap()[j4 * 4 * P:(j4 + 1) * 4 * P, :].rearrange(
                    "(o p) f -> p o f", p=P))
            nc.vector.tensor_scalar(nb[:, j4 * 4:(j4 + 1) * 4], atj[:],
                                    0.5, NEGB, OP.is_le, OP.mult)
        xp_ctx = tc.tile_pool(name="xp", bufs=1)
        xp = xp_ctx.__enter__()
        xt = xp.tile([P, NIN // P, R], F32R, tag="xt")
        nc.sync.dma_start(xt[:], xT.ap().rearrange("(o p) f -> p o f", p=P))

        # resident activations
        t1t = glob.tile([P, E1 // P, R], F32R, tag="t1t")
        t2t = glob.tile([P, E2 // P, R], F32R, tag="t2t")
        zt = glob.tile([P, R], F32R, tag="zt")
        h1ot = glob.tile([P, E1 // P, R], F32, tag="h1ot")
        bl5 = glob.tile([P, R], F32, tag="bl5")
        nrm2 = glob.tile([1, R], F32, tag="nrm2")

        t3_dram = dr.tile([E3, R], F32R, name="t3_dram")
        d1_dram = dr.tile([2048, R], F32R, name="d1_dram")
        bl4_dram = dr.tile([E3, R], F32, name="bl4_dram")
        pt_dram = dr.tile([N, R], F32R, name="pt_dram")

        def proj_res(rhs, kcn, w, D, out_cb):
            """mc-outer projection with SBUF-resident rhs [P, kcn, R]."""
            for mc in range(_cdiv(D, P)):
                m = min(P, D - mc * P)
                pp = psum("pr")
                for kc in range(kcn):
                    nc.tensor.matmul(pp[:m], w[:, kc, mc * P:mc * P + m],
                                     rhs[:, kc], start=(kc == 0),
                                     stop=(kc == kcn - 1),
                                     skip_group_check=True)
                out_cb(mc, m, pp)

        def proj_stream(src_dram, kcn, w, D, out_cb, src_dt=F32R, nm="pk"):
            """kc-outer projection streaming rhs chunks (4 per DMA) from DRAM."""
            mcn = _cdiv(D, P)
            pps = [psum(f"{nm}{mc}") for mc in range(mcn)]
            g = 4
            for k4 in range(_cdiv(kcn, g)):
                kn = min(g, kcn - k4 * g)
                ch = ttile(f"{nm}ch{k4}", tag="mt4", bufs=2, shape=[P, g, R], dtype=src_dt)
                nc.sync.dma_start(
                    ch[:, :kn],
                    src_dram[k4 * g * P:(k4 * g + kn) * P, :].rearrange(
                        "(o p) f -> p o f", p=P))
                for ki in range(kn):
                    kc = k4 * g + ki
                    for mc in range(mcn):
                        m = min(P, D - mc * P)
                        nc.tensor.matmul(
                            pps[mc][:m], w[:, kc, mc * P:mc * P + m],
                            ch[:, ki], start=(kc == 0), stop=(kc == kcn - 1),
                            skip_group_check=True)
            for mc in range(mcn):
                out_cb(mc, min(P, D - mc * P), pps[mc])

        # ---------- AE ----------
        with nc.named_scope("ae"):
            w = load_w("enc1_w", NIN, E1); b = load_bias("enc1_b", E1)
            proj_res(xt, NIN // P, w, E1, lambda mc, m, pp: nc.scalar.activation(
                t1t[:m, mc], pp[:m], AF.Relu, bias=b[:m, mc:mc + 1]))
            w = load_w("enc2_w", E1, E2); b = load_bias("enc2_b", E2)
            proj_res(t1t, E1 // P, w, E2, lambda mc, m, pp: nc.scalar.activation(
                t2t[:m, mc], pp[:m], AF.Relu, bias=b[:m, mc:mc + 1]))
            w = load_w("enc3_w", E2, E3); b = load_bias("enc3_b", E3)

            def t3_cb(mc, m, pp, b=b):
                t3c = ttile(f"t3c{mc}", dtype=F32R)
                nc.scalar.activation(t3c[:m], pp[:m], AF.Relu,
                                     bias=b[:m, mc:mc + 1])
                nc.sync.dma_start(t3_dram[mc * P:mc * P + m, :], t3c[:m])
            proj_res(t2t, E2 // P, w, E3, t3_cb)

            w = load_w("zl_w", E3, NZ); b = load_bias("zl_b", NZ)

            def zl_cb(mc, m, pp, b=b):
                nc.scalar.activation(zt[:NZ], pp[:NZ], AF.Identity,
                                     bias=b[:NZ, 0:1])
            proj_stream(t3_dram, E3 // P, w, NZ, zl_cb, nm="zl")
            nc.sync.dma_start(zt[NZ:, :], dt_in["zer64r"].ap())
            nc.sync.dma_start(zT_o.ap(), zt[:NZ].bitcast(F32))

            w = load_w("dec1_w", P, 2048); b = load_bias("dec1_b", 2048)
            zt3 = zt[:, None, :]

            def d1_cb(mc, m, pp, b=b):
                d1c = ttile(f"d1c{mc}", dtype=F32R)
                nc.scalar.activation(d1c[:m], pp[:m], AF.Relu,
                                     bias=b[:m, mc:mc + 1])
                nc.sync.dma_start(d1_dram[mc * P:mc * P + m, :], d1c[:m])
            proj_res(zt3, 1, w, 2048, d1_cb)

            d2t = wp.tile([P, E2 // P, R], F32R, tag="d2t")
            w = load_w("dec2_w", 2048, E2); b = load_bias("dec2_b", E2)
            proj_stream(d1_dram, 2048 // P, w, E2,
                        lambda mc, m, pp, b=b: nc.scalar.activation(
                            d2t[:m, mc], pp[:m], AF.Relu, bias=b[:m, mc:mc + 1]),
                        nm="d2")
            d3t = wp.tile([P, 512 // P, R], F32R, tag="d3t")
            w = load_w("dec3_w", E2, 512); b = load_bias("dec3_b", 512)
            proj_res(d2t, E2 // P, w, 512, lambda mc, m, pp, b=b:
                     nc.scalar.activation(d3t[:m, mc], pp[:m], AF.Relu,
                                          bias=b[:m, mc:mc + 1]))
            w = load_w("xbar_w", 512, NIN); b = load_bias("xbar_b", NIN)

            def xb_cb(mc, m, pp, b=b):
                xbc = ttile(f"xbc{mc}")
                nc.scalar.activation(xbc[:m], pp[:m], AF.Identity,
                                     bias=b[:m, mc:mc + 1])
                nc.sync.dma_start(xbarT_o.ap()[mc * P:mc * P + m, :], xbc[:m])
            proj_res(d3t, 512 // P, w, NIN, xb_cb)

        # ---------- GAT helpers ----------
        def gat_proj_gather(lname, rhs, kcn, wname, d, asn, ann, vdt,
                            h_full=None, wdt=None):
            """Project h (chunkwise), accumulate f_self/f_neigh, transpose each
            chunk into the row-major DRAM bounce, gather. Returns
            (hg, fs_tile, bo)."""
            mcn = _cdiv(d, P)
            w = load_w(wname, kcn * P, d, wdt if wdt is not None else vdt)
            asv = load_vec(asn, d, vdt)
            anv = load_vec(ann, d, vdt)
            fs_ps, fn_ps = psum("fs"), psum("fn")
            bo = dr.tile([R, d + 8], F32R if vdt == F32R else F32,
                         name=f"bo_{lname}")
            idn = identf if vdt == F32 else identr

            for mc in range(mcn):
                m = min(P, d - mc * P)
                pp = psum("hp")
                for kc in range(kcn):
                    nc.tensor.matmul(pp[:m], w[:, kc, mc * P:mc * P + m],
                                     rhs[:, kc], start=(kc == 0),
                                     stop=(kc == kcn - 1),
                                     skip_group_check=True)
                if h_full is not None:
                    hc = h_full[:, mc]
                    nc.vector.tensor_copy(hc, pp[:m])
                else:
                    hc = ttile(f"hc_{lname}{mc}", tag="hc", bufs=2, dtype=vdt)
                    nc.vector.tensor_copy(hc[:m], pp[:m])
                    hc = hc[:m]
                nc.tensor.matmul(fs_ps[:1], asv[:, mc], hc,
                                 start=(mc == 0), stop=(mc == mcn - 1),
                                 skip_group_check=True)
                nc.tensor.matmul(fn_ps[:1], anv[:, mc], hc,
                                 start=(mc == 0), stop=(mc == mcn - 1),
                                 skip_group_check=True)
                blk = ttile(f"bk_{lname}{mc}", tag="blk4", bufs=2,
                            dtype=bo.dtype, shape=[P, RC, P])
                for ns in range(RC):
                    tpp = psum("tp", dtype=vdt)
                    nc.tensor.transpose(tpp[:, :P], hc[:, ns * P:(ns + 1) * P],
                                        idn[:])
                    nc.vector.tensor_copy(blk[:, ns].bitcast(F32),
                                          tpp[:, :P].bitcast(F32))
                nc.sync.dma_start(
                    bo[:, mc * P:mc * P + m].rearrange("(o p) f -> p o f", p=P),
                    blk[:, :, :m])

            fs_t = ttile(f"fs_{lname}", tag="frow", bufs=4, shape=[1, R])
            nc.scalar.copy(fs_t[:], fs_ps[:1])
            fn_t = ttile(f"fn_{lname}", tag="frow", bufs=4, shape=[1, R])
            nc.scalar.copy(fn_t[:], fn_ps[:1])
            bocol = bo[:, d:d + 1]
            if bo.dtype != F32:
                bocol = bocol.bitcast(F32)
            nc.sync.dma_start(bocol.rearrange("f p -> p f"), fn_t[:])
            hg = dr.tile([NCORES, R, d + 8], bo.dtype, name=f"hg_{lname}",
                         addr_space="Shared")
            nc.gpsimd.collective_compute(
                "AllGather", OP.bypass, replica_groups=rg,
                ins=[bo.opt()], outs=[hg.opt()])
            return hg, fs_t

        def gat_attention(lname, hg, d, fs_t, exact, epi, spill):
            """Fused e-chain + att@h. epi(mc, m, y) gets normalized out.T
            chunks. spill=True: 3-pass p spill via pt_dram (for d=2048)."""
            mcn = _cdiv(d, P)
            hdt = F32 if exact else F32R
            FS = ttile(f"FS_{lname}", tag="pfull", bufs=3)
            nc.gpsimd.partition_broadcast(FS[:], fs_t[:])
            # all f_neigh columns in one tile: fncols[p, jc] = fn[jc*128+p]
            fncols = tp.tile([P, JC], F32, tag="fncols", bufs=2,
                             name=f"fnc_{lname}")
            for cc in range(NCORES):
                csrc = hg[cc, :, d:d + 1]
                if csrc.dtype != F32:
                    csrc = csrc.bitcast(F32)
                nc.sync.dma_start(
                    fncols[:, cc * RC:(cc + 1) * RC],
                    csrc.rearrange("(o p) one -> p (o one)", p=P))
            if spill:
                passes = [list(range(0, 7)), list(range(7, 15)), [15]]
            else:
                passes = [list(range(mcn))]
            rb = None
            for pi, mcs in enumerate(passes):
                c0, cn = mcs[0] * P, min(d, (mcs[-1] + 1) * P) - mcs[0] * P
                pos = {mc: psum(f"po_{lname}{mc}") for mc in mcs}
                rs_ps = psum("rs") if pi == 0 else None
                for jc in range(JC):
                    cc, rr_ = jc // RC, jc % RC
                    if pi == 0:
                        if jc % 4 == 0:
                            mtj4 = ttile(f"mt_{lname}{jc}", tag="mt4", bufs=2,
                                         shape=[P, 4, R])
                            nc.sync.dma_start(
                                mtj4[:],
                                MT.ap()[jc * P:(jc + 4) * P, :].rearrange(
                                    "(o p) f -> p o f", p=P))
                        u = ttile(f"u_{lname}{jc}")
                        nc.vector.scalar_tensor_tensor(
                            u[:], FS[:], fncols[:, jc:jc + 1], mtj4[:, jc % 4],
                            OP.add, OP.mult)
                        el = ttile(f"el_{lname}{jc}")
                        nc.scalar.activation(el[:], u[:], AF.Prelu, alpha=ALPHA)
                        em = ttile(f"em_{lname}{jc}")
                        nc.vector.tensor_add(em[:], el[:], nb[:, jc])
                        pch = ttile(f"pch_{lname}{jc}", tag="pch", bufs=3,
                                    dtype=F32R)
                        nc.scalar.activation(pch[:], em[:], AF.Exp)
                        if spill:
                            nc.sync.dma_start(
                                pt_dram[jc * P:(jc + 1) * P, :], pch[:])
                        nc.tensor.matmul(rs_ps[:1], ones_r[:], pch[:],
                                         start=(jc == 0), stop=(jc == JC - 1),
                                         skip_group_check=True)
                    else:
                        pch = ttile(f"pch_{lname}{pi}_{jc}", tag="pch", bufs=3,
                                    dtype=F32R)
                        nc.sync.dma_start(pch[:],
                                          pt_dram[jc * P:(jc + 1) * P, :])
                    rhs = pch[:].bitcast(F32) if exact else pch[:]
                    # one slab DMA covering this pass's mc-columns
                    hb = tp.tile([P, cn], hdt, tag="hbslab", bufs=2,
                                 name=f"hb_{lname}{pi}_{jc}")
                    hsrc = hg[cc, rr_ * P:(rr_ + 1) * P, c0:c0 + cn]
                    if hsrc.dtype != hdt:
                        hsrc = hsrc.bitcast(hdt)
                    nc.sync.dma_start(hb[:], hsrc)
                    for mc in mcs:
                        m = min(P, d - mc * P)
                        o0 = mc * P - c0
                        nc.tensor.matmul(pos[mc][:m], hb[:, o0:o0 + m], rhs,
                                         start=(jc == 0), stop=(jc == JC - 1),
                                         skip_group_check=True)
                if pi == 0:
                    rs = ttile(f"rs_{lname}", tag="frow", bufs=4, shape=[1, R])
                    nc.scalar.copy(rs[:], rs_ps[:1])
                    rr2 = ttile(f"rr_{lname}", tag="frow", bufs=4,
                                shape=[1, R])
                    nc.vector.reciprocal(rr2[:], rs[:])
                    rb = ttile(f"rb_{lname}", tag="pfull", bufs=3)
                    nc.gpsimd.partition_broadcast(rb[:], rr2[:])
                for mc in mcs:
                    m = min(P, d - mc * P)
                    y = ttile(f"y_{lname}{mc}")
                    nc.vector.tensor_mul(y[:m], pos[mc][:m], rb[:m])
                    epi(mc, m, y)

        def elu_c1(lname, mc, m, y):
            mn = ttile(f"mn_{lname}{mc}")
            nc.vector.tensor_scalar_min(mn[:m], y[:m], 0.0)
            ex = ttile(f"ex_{lname}{mc}")
            nc.scalar.activation(ex[:m], mn[:m], AF.Exp)
            c1 = ttile(f"c1_{lname}{mc}")
            nc.vector.scalar_tensor_tensor(
                c1[:m], y[:m], 0.0, ex[:m], OP.max, OP.add)
            return c1  # relu(y) + exp(min(y,0)) ; elu = c1 - 1

        def blend_ip(lname, dst, tsrc, c1, mc, m):
            # dst = 0.9*(c1-1) + 0.1*tsrc
            c2 = ttile(f"c2_{lname}{mc}")
            nc.vector.tensor_scalar(c2[:m], c1[:m], -1.0, 0.9, OP.add, OP.mult)
            nc.vector.scalar_tensor_tensor(dst, tsrc, 0.1, c2[:m],
                                           OP.mult, OP.add)

        # ---------- g1 ----------
        with nc.named_scope("g1"):
            hg1, fs1 = gat_proj_gather("g1", xt, NIN // P, "g1_w", E1,
                                       "as1", "an1", F32R)
            xp_ctx.__exit__(None, None, None)

            def epi1(mc, m, y):
                c1 = elu_c1("g1", mc, m, y)
                # h1 (gat output) kept for the s-diffusion chain
                nc.vector.tensor_scalar_add(h1ot[:m, mc], c1[:m], -1.0)
                c2 = ttile(f"c2g1_{mc}")
                nc.vector.tensor_scalar_mul(c2[:m], h1ot[:m, mc], 0.9)
                nc.vector.scalar_tensor_tensor(
                    t1t[:m, mc], t1t[:m, mc].bitcast(F32), 0.1, c2[:m],
                    OP.mult, OP.add)
            gat_attention("g1", hg1, E1, fs1, False, epi1, False)

            # gather h1 (gat output, feature-major) + per-node sq-norms row
            nr_ps = psum("nr")
            for mc in range(E1 // P):
                sqc = ttile(f"sq{mc}")
                nc.scalar.activation(sqc[:], h1ot[:, mc], AF.Square)
                nc.tensor.matmul(nr_ps[:1], ones_f[:], sqc[:],
                                 start=(mc == 0), stop=(mc == E1 // P - 1),
                                 skip_group_check=True)
            nc.scalar.copy(nrm2[:], nr_ps[:1])
            bo1o = dr.tile([E1 + 8, R], F32, name="bo1o")
            nc.sync.dma_start(bo1o[:E1].rearrange("(o p) f -> p o f", p=P),
                              h1ot[:])
            nc.sync.dma_start(bo1o[E1:E1 + 1, :], nrm2[:])
            hg1o = dr.tile([NCORES, E1 + 8, R], F32, name="hg1o",
                           addr_space="Shared")
            nc.gpsimd.collective_compute(
                "AllGather", OP.bypass, replica_groups=rg,
                ins=[bo1o.opt()], outs=[hg1o.opt()])

        # ---------- g2 ----------
        with nc.named_scope("g2"):
            hg2, fs2 = gat_proj_gather("g2", t1t, E1 // P, "g2_w", E2,
                                       "as2", "an2", F32R)

            def epi2(mc, m, y):
                c1 = elu_c1("g2", mc, m, y)
                blend_ip("g2", t2t[:m, mc], t2t[:m, mc].bitcast(F32), c1, mc, m)
            gat_attention("g2", hg2, E2, fs2, False, epi2, False)

        # ---------- g3 ----------
        with nc.named_scope("g3"):
            hg3, fs3 = gat_proj_gather("g3", t2t, E2 // P, "g3_w", E3,
                                       "as3", "an3", F32R)

            def epi3(mc, m, y):
                c1 = elu_c1("g3", mc, m, y)
                t3e = ttile(f"t3e{mc}", dtype=F32R)
                nc.sync.dma_start(t3e[:m], t3_dram[mc * P:mc * P + m, :])
                c2 = ttile(f"c23_{mc}")
                nc.vector.tensor_scalar(c2[:m], c1[:m], -1.0, 0.9,
                                        OP.add, OP.mult)
                bl = ttile(f"bl3_{mc}")
                nc.vector.scalar_tensor_tensor(
                    bl[:m], t3e[:m].bitcast(F32), 0.1, c2[:m],
                    OP.mult, OP.add)
                nc.sync.dma_start(bl4_dram[mc * P:mc * P + m, :], bl[:m])
            gat_attention("g3", hg3, E3, fs3, False, epi3, True)

        # ---------- g4 (exact fp32) ----------
        with nc.named_scope("g4"):
            w4 = load_w("g4_w", E3, NZ, F32)
            h4t = ttile("h4t", tag="pfull", bufs=3)
            bo4 = dr.tile([R, NZ + 8], F32, name="bo_g4")
            p4t = psum("h4t")
            p4r = [psum(f"h4r{ns}") for ns in range(RC)]
            for kc in range(E3 // P):
                blk4 = ttile(f"bl4k{kc}")
                nc.sync.dma_start(blk4[:], bl4_dram[kc * P:(kc + 1) * P, :])
                nc.tensor.matmul(p4t[:NZ], w4[:, kc, :NZ], blk4[:],
                                 start=(kc == 0), stop=(kc == E3 // P - 1),
                                 skip_group_check=True)
                for ns in range(RC):
                    nc.tensor.matmul(p4r[ns][:, :NZ],
                                     blk4[:, ns * P:(ns + 1) * P],
                                     w4[:, kc, :NZ], start=(kc == 0),
                                     stop=(kc == E3 // P - 1),
                                     skip_group_check=True)
            nc.vector.tensor_copy(h4t[:NZ], p4t[:NZ])
            nc.sync.dma_start(h4t[NZ:], dt_in["zer64f"].ap())
            for ns in range(RC):
                h4rc = tp.tile([P, NZ], F32, tag="blk", bufs=6,
                               name=f"h4rc{ns}")
                nc.vector.tensor_copy(h4rc[:], p4r[ns][:, :NZ])
                nc.sync.dma_start(bo4[ns * P:(ns + 1) * P, :NZ], h4rc[:])
            asv4 = cst.tile([P, 1], F32, tag="as4")
            nc.sync.dma_start(asv4[:], dt_in["as4"].ap())
            anv4 = cst.tile([P, 1], F32, tag="an4")
            nc.sync.dma_start(anv4[:], dt_in["an4"].ap())
            pf4 = psum("fs4")
            nc.tensor.matmul(pf4[:1], asv4[:], h4t[:], start=True, stop=True,
                             skip_group_check=True)
            fs4 = ttile("fs4", tag="frow", bufs=4, shape=[1, R])
            nc.scalar.copy(fs4[:], pf4[:1])
            pn4 = psum("fn4")
            nc.tensor.matmul(pn4[:1], anv4[:], h4t[:], start=True, stop=True,
                             skip_group_check=True)
            fn4 = ttile("fn4", tag="frow", bufs=4, shape=[1, R])
            nc.scalar.copy(fn4[:], pn4[:1])
            nc.sync.dma_start(bo4[:, NZ:NZ + 1].rearrange("f p -> p f"),
                              fn4[:])
            hg4 = dr.tile([NCORES, R, NZ + 8], F32, name="hg_g4",
                          addr_space="Shared")
            nc.gpsimd.collective_compute(
                "AllGather", OP.bypass, replica_groups=rg,
                ins=[bo4.opt()], outs=[hg4.opt()])

            zt01 = ttile("zt01", tag="pfull", bufs=3)
            nc.vector.tensor_scalar_mul(zt01[:NZ], zt[:NZ].bitcast(F32), 0.1)

            def epi4(mc, m, y):
                c1 = elu_c1("g4", mc, m, y)
                h4o = ttile("h4o")
                nc.vector.tensor_scalar_add(h4o[:m], c1[:m], -1.0)
                nc.sync.dma_start(h4T_o.ap(), h4o[:m])
                nc.vector.scalar_tensor_tensor(
                    bl5[:m], h4o[:m], 0.9, zt01[:m], OP.mult, OP.add)
            gat_attention("g4", hg4, NZ, fs4, True, epi4, False)
            nc.sync.dma_start(bl5[NZ:], dt_in["zer64f"].ap())

        # ---------- GNN + s-diffusion ----------
        with nc.named_scope("gnn_s"):
            w5 = cst.tile([P, NCL], F32, tag="g5w")
            nc.sync.dma_start(w5[:], dt_in["g5w"].ap())
            pw = psum("wg")
            nc.tensor.matmul(pw[:NCL], w5[:], bl5[:], start=True, stop=True,
                             skip_group_check=True)
            wgT = ttile("wgT", tag="c16", bufs=4, shape=[NCL, R])
            nc.scalar.copy(wgT[:], pw[:NCL])
            bw = dr.tile([R, NCL], F32, name="bw")
            nc.sync.dma_start(bw[:].rearrange("n c -> c n"), wgT[:])
            wg_row = dr.tile([NCORES, R, NCL], F32, name="wg_row",
                             addr_space="Shared")
            nc.gpsimd.collective_compute(
                "AllGather", OP.bypass, replica_groups=rg,
                ins=[bw.opt()], outs=[wg_row.opt()])

            ph = psum("hgnn")
            for jc in range(JC):
                cc, rr_ = jc // RC, jc % RC
                if rr_ == 0:
                    wrb4 = tp.tile([P, RC, NCL], F32, tag="blk4", bufs=2,
                                   name=f"wrb{cc}")
                    nc.sync.dma_start(
                        wrb4[:], wg_row[cc].rearrange("(o p) c -> p o c", p=P))
                if jc % 4 == 0:
                    atj4 = ttile(f"adj2_{jc}", tag="mt4", bufs=2,
                                 shape=[P, 4, R])
                    nc.sync.dma_start(
                        atj4[:], adjT.ap()[jc * P:(jc + 4) * P, :].rearrange(
                            "(o p) f -> p o f", p=P))
                adm = ttile(f"adm{jc}")
                nc.vector.scalar_tensor_tensor(
                    adm[:], atj4[:, jc % 4], 0.5, atj4[:, jc % 4],
                    OP.is_gt, OP.mult)
                nc.tensor.matmul(ph[:NCL], wrb4[:, rr_], adm[:],
                                 start=(jc == 0), stop=(jc == JC - 1),
                                 skip_group_check=True)
            hgt = ttile("hgt", tag="c16", bufs=4, shape=[NCL, R])
            nc.scalar.copy(hgt[:], ph[:NCL])
            bh = dr.tile([R, NCL], F32, name="bh")
            nc.sync.dma_start(bh[:].rearrange("n c -> c n"), hgt[:])
            hg_gnn = dr.tile([NCORES, R, NCL], F32, name="hg_gnn",
                             addr_space="Shared")
            nc.gpsimd.collective_compute(
                "AllGather", OP.bypass, replica_groups=rg,
                ins=[bh.opt()], outs=[hg_gnn.opt()])

            # K_i = sqrt(nrm2_i) * maxnorm for a safe exp
            mx = tp.tile([1, 1], F32, tag="mx", bufs=1, name="mx")
            mxc = tp.tile([1, 1], F32, tag="mxc", bufs=8, name="mxc")
            for c in range(NCORES):
                nr_c = ttile(f"nr{c}", tag="frow", bufs=4, shape=[1, R])
                nc.sync.dma_start(nr_c[:], hg1o[c, E1:E1 + 1, :])
                red = tp.tile([1, 1], F32, tag="mxc", bufs=8, name=f"red{c}")
                nc.vector.tensor_reduce(red[:], nr_c[:], mybir.AxisListType.X,
                                        OP.max)
                if c == 0:
                    nc.vector.tensor_copy(mx[:], red[:])
                else:
                    nc.vector.tensor_tensor(mx[:], mx[:], red[:], OP.max)
            mxs = tp.tile([1, 1], F32, tag="mxs", bufs=1, name="mxs")
            nc.scalar.activation(mxs[:], mx[:], AF.Sqrt)
            kr = ttile("kr", tag="frow", bufs=4, shape=[1, R])
            nc.scalar.activation(kr[:], nrm2[:], AF.Sqrt)
            nc.vector.tensor_scalar(kr[:], kr[:], mxs[:], None, OP.mult)
            kb = ttile("kb", tag="pfull", bufs=3)
            nc.gpsimd.partition_broadcast(kb[:], kr[:])

            pdn = psum("dn")
            pz3 = psum("z3")
            for jc in range(JC):
                cc, rr_ = jc // RC, jc % RC
                pes = psum(f"es{jc}")
                h1b = tp.tile([P, E1 // P, P], F32, tag="h1b4", bufs=2,
                              name=f"h1b_{jc}")
                nc.sync.dma_start(
                    h1b[:], hg1o[cc, :E1, rr_ * P:(rr_ + 1) * P].rearrange(
                        "(o p) f -> p o f", p=P))
                for kc in range(E1 // P):
                    nc.tensor.matmul(pes[:], h1b[:, kc], h1ot[:, kc],
                                     start=(kc == 0), stop=(kc == E1 // P - 1),
                                     skip_group_check=True)
                esm = ttile(f"esm{jc}")
                nc.vector.tensor_tensor(esm[:], pes[:], kb[:], OP.subtract)
                psc = ttile(f"psc{jc}")
                nc.scalar.activation(psc[:], esm[:], AF.Exp)
                nc.tensor.matmul(pdn[:1], ones_f[:], psc[:], start=(jc == 0),
                                 stop=(jc == JC - 1), skip_group_check=True)
                if rr_ == 0:
                    hgb4 = tp.tile([P, RC, NCL], F32, tag="blk4", bufs=2,
                                   name=f"hgb{cc}")
                    nc.sync.dma_start(
                        hgb4[:], hg_gnn[cc].rearrange("(o p) c -> p o c", p=P))
                nc.tensor.matmul(pz3[:NCL], hgb4[:, rr_], psc[:],
                                 start=(jc == 0), stop=(jc == JC - 1),
                                 skip_group_check=True)
            dnb = ttile("dnb", tag="frow", bufs=4, shape=[1, R])
            nc.scalar.copy(dnb[:], pdn[:1])
            dnr = ttile("dnr", tag="frow", bufs=4, shape=[1, R])
            nc.vector.reciprocal(dnr[:], dnb[:])
            drb = ttile("drb", tag="c16", bufs=4, shape=[NCL, R])
            nc.gpsimd.partition_broadcast(drb[:], dnr[:], channels=NCL)
            z3n = ttile("z3n", tag="c16", bufs=4, shape=[NCL, R])
            nc.vector.tensor_mul(z3n[:], pz3[:NCL], drb[:])

            mxp = ttile("mxp", tag="c16", bufs=4, shape=[NCL, R])
            nc.gpsimd.partition_all_reduce(mxp[:], z3n[:], NCL,
                                           bass_isa.ReduceOp.max)
            zc = ttile("zc", tag="c16", bufs=4, shape=[NCL, R])
            nc.vector.tensor_tensor(zc[:], z3n[:], mxp[:], OP.subtract)
            pe_ = ttile("pe", tag="c16", bufs=4, shape=[NCL, R])
            nc.scalar.activation(pe_[:], zc[:], AF.Exp)
            se_ = ttile("se", tag="c16", bufs=4, shape=[NCL, R])
            nc.gpsimd.partition_all_reduce(se_[:], pe_[:], NCL,
                                           bass_isa.ReduceOp.add)
            sre = ttile("sre", tag="c16", bufs=4, shape=[NCL, R])
            nc.vector.reciprocal(sre[:], se_[:])
            prd = ttile("prd", tag="c16", bufs=4, shape=[NCL, R])
            nc.vector.tensor_mul(prd[:], pe_[:], sre[:])
            nc.sync.dma_start(predT_o.ap(), prd[:])

        # ---------- student-t q ----------
        with nc.named_scope("studentt"):
            clu = cst.tile([P, NCL], F32R, tag="cluT")
            nc.sync.dma_start(clu[:], dt_in["cluT"].ap())
            pcr = psum("cross")
            nc.tensor.matmul(pcr[:NCL], clu[:], zt[:], start=True, stop=True,
                             skip_group_check=True)
            zsq = ttile("zsq")
            nc.scalar.activation(zsq[:], zt[:].bitcast(F32), AF.Square)
            pzn = psum("zn")
            nc.tensor.matmul(pzn[:1], ones_f[:], zsq[:], start=True, stop=True,
                             skip_group_check=True)
            znl = ttile("znl", tag="frow", bufs=4, shape=[1, R])
            nc.scalar.copy(znl[:], pzn[:1])
            znb = ttile("znb", tag="c16", bufs=4, shape=[NCL, R])
            nc.gpsimd.partition_broadcast(znb[:], znl[:], channels=NCL)
            d2 = ttile("d2", tag="c16", bufs=4, shape=[NCL, R])
            nc.vector.scalar_tensor_tensor(
                d2[:], pcr[:NCL], -2.0, znb[:], OP.mult, OP.add)
            nc.vector.tensor_scalar(d2[:], d2[:], clun2[:], 1.0,
                                    OP.add, OP.add)
            qn = ttile("qn", tag="c16", bufs=4, shape=[NCL, R])
            nc.vector.reciprocal(qn[:], d2[:])
            qs = ttile("qs", tag="c16", bufs=4, shape=[NCL, R])
            nc.gpsimd.partition_all_reduce(qs[:], qn[:], NCL,
                                           bass_isa.ReduceOp.add)
            qsr = ttile("qsr", tag="c16", bufs=4, shape=[NCL, R])
            nc.vector.reciprocal(qsr[:], qs[:])
            qf = ttile("qf", tag="c16", bufs=4, shape=[NCL, R])
            nc.vector.tensor_mul(qf[:], qn[:], qsr[:])
            nc.sync.dma_start(qT_o.ap(), qf[:])

    nc.compile()
    return nc


def _prep_maps(x, adj, M, params):
    p = {k: np.ascontiguousarray(np.asarray(v, dtype=np.float32))
         for k, v in params.items()}
    x = np.asarray(x, dtype=np.float32)
    adj = np.asarray(adj, dtype=np.float32)
    M = np.asarray(M, dtype=np.float32)

    shared = {}
    for nm in ["enc1", "enc2", "enc3", "zl", "dec2", "dec3", "xbar"]:
        shared[nm + "_w"] = p[nm + "_w"]
    shared["dec1_w"] = np.pad(p["dec1_w"], ((0, P - NZ), (0, 0)))
    for nm, do in [("enc1", E1), ("enc2", E2), ("enc3", E3), ("zl", NZ),
                   ("dec1", 2048), ("dec2", E2), ("dec3", 512), ("xbar", NIN)]:
        b = p[nm + "_b"]
        if do >= P:
            shared[nm + "_b"] = np.ascontiguousarray(b.reshape(-1, P).T)
        else:
            bb = np.zeros((P, 1), np.float32)
            bb[:do, 0] = b
            shared[nm + "_b"] = bb
    for i in (1, 2, 3, 4):
        shared[f"g{i}_w"] = p[f"g{i}_w"]
        a_s, a_n = p[f"g{i}_as"], p[f"g{i}_an"]
        if a_s.shape[0] < P:
            a_s = np.pad(a_s, ((0, P - a_s.shape[0]), (0, 0)))
            a_n = np.pad(a_n, ((0, P - a_n.shape[0]), (0, 0)))
        shared[f"as{i}"] = a_s
        shared[f"an{i}"] = a_n
    shared["g5w"] = np.pad(p["g5_w"], ((0, P - NZ), (0, 0)))
    shared["cluT"] = np.pad(np.ascontiguousarray(p["cluster"].T),
                            ((0, P - NZ), (0, 0)))
    shared["clun2"] = np.ascontiguousarray(
        (p["cluster"] ** 2).sum(axis=1, keepdims=True))
    shared["identf"] = np.eye(P, dtype=np.float32)
    shared["identr"] = np.eye(P, dtype=np.float32)
    shared["ones_r"] = np.ones((P, 1), np.float32)
    shared["ones_f"] = np.ones((P, 1), np.float32)
    shared["zer64r"] = np.zeros((NZ, R), np.float32)
    shared["zer64f"] = np.zeros((NZ, R), np.float32)

    in_maps = []
    for c in range(NCORES):
        r0, r1 = c * R, (c + 1) * R
        m = dict(shared)
        m["xT"] = np.ascontiguousarray(x[r0:r1].T)
        m["adjT"] = np.ascontiguousarray(adj[r0:r1].T)
        m["MT"] = np.ascontiguousarray(M[r0:r1].T)
        in_maps.append(m)
    return in_maps


def kernel(x, adj, M, params):
    if "nc" not in _CACHED:
        _CACHED["nc"] = build_nc()
    nc = _CACHED["nc"]
    in_maps = _prep_maps(x, adj, M, params)
    res = run_bass_kernel_spmd(nc, in_maps, core_ids=list(range(NCORES)))
    xb = np.concatenate([r["xbarT"].T for r in res.results], axis=0)
    q = np.concatenate([r["qT"].T for r in res.results], axis=0)
    pred = np.concatenate([r["predT"].T for r in res.results], axis=0)
    z = np.concatenate([r["zT"].T for r in res.results], axis=0)
    h4 = np.concatenate([r["h4T"].T for r in res.results], axis=0)
    return (xb, q, pred, z, h4)
